# revision 21
# baseline (speedup 1.0000x reference)
"""Trainium2 Bass kernel for nn_EnhancedOFTOutputLayer.

Math (per reference):
    S = 0.5*(A - A^T) per block (A = proj_R[b], 512x512, S skew-symmetric)
    Q = (I - S) @ inv(I + S + 1e-6 I)          (Cayley, orthogonal)
    filt = blockdiag(Q) @ weight               (block-row matmuls)
    y = x @ filt^T + bias

Sharding: tensor-parallel over the 8 blocks -> core b owns output rows
[512b, 512b+512).  x^T is replicated; each core computes
y_b^T = filt_b @ x^T  ([512, 8192]) with no cross-core communication.

Cayley inverse per core via Newton-Schulz (||S||_2 ~ 0.64 here, so
quadratic convergence; 4 iterations reach the arithmetic floor).  All
iterates are polynomials in the skew matrix S, so they commute and
P(S)^T = P(-S).  That gives a 3-product iteration with every stationary
operand available pre-transposed (no PE transposes):
    T1t = X^T D^T           = mm(lhsT=X,   rhs=Dt)     (= (D X)^T)
    Xn  = 2X  - (DX)X       = mm(lhsT=T1t, rhs=X),  post 2X - ps
    Xnt = 2Xt - ((DX)X)^T   = mm(lhsT=X,   rhs=T1t), post 2Xt - ps
and finally Q^T = N @ X = mm(lhsT=Nt, rhs=X), Nt = I - S.

Matmuls run in float32r (PE 1 cyc/row vs 4 for fp32; rel err ~3e-4,
far inside the 2e-2 gate).  PSUM accumulation is fp32.  fp32r rounding
happens in SWDGE cast-DMAs or DVE copies.

Memory layout is arranged so weight + the first x tile live in the
persistent pool: their DMAs have no WAR hazard against the Cayley
scratch (the stack allocator reuses closed-pool space), letting them
prefetch during the Newton iterations.  x ingestion is hybrid
(28 i-chunks SWDGE cast-DMA + 4 via HWDGE fp32 and DVE round) so
neither DMA path limits the PE.

Host-side prep is layout-only: per-block slicing, transposes, and
re-tiling so every DMA reads one contiguous run per partition.
"""

import numpy as np

import concourse.bass as bass
import concourse.mybir as mybir
import concourse.tile as tile
from concourse import bacc
from concourse.bass_utils import run_bass_kernel_spmd

HID = 4096
NB = 8
BS = 512  # block size
NTOK = 8192  # 4*2048
P = 128
BC = BS // P  # 4 row-chunks per 512-mat
IC = HID // P  # 32 i-chunks
ICH = 28  # i-chunks via SWDGE cast-DMA; the rest via HWDGE + DVE round
TCH = 256  # token chunk (matmul moving free dim; fp32r needs >=256)
NT = NTOK // TCH
NEWTON_ITERS = 4
IGR = 4  # i-chunks per wb load group
F32 = mybir.dt.float32
F32R = mybir.dt.float32r

_CACHE = {}


def _build():
    nc = bacc.Bacc("TRN2", target_bir_lowering=False)

    # all host-pretiled to [P, ...contiguous...] so DMAs are slab reads
    wb_d = nc.dram_tensor("wbl", [P, BC, HID], F32, kind="ExternalInput")
    pa_d = nc.dram_tensor("pal", [P, BC, BS], F32, kind="ExternalInput")
    pat_d = nc.dram_tensor("patl", [P, BC, BS], F32, kind="ExternalInput")
    eye_d = nc.dram_tensor("eyel", [P, BC, BS], F32, kind="ExternalInput")
    bias_d = nc.dram_tensor("bias2d", [P, BC], F32, kind="ExternalInput")
    xt_d = nc.dram_tensor("xtl", [NT, P, IC, TCH], F32, kind="ExternalInput")
    yt_d = nc.dram_tensor("ytl", [NT, P, BC, TCH], F32, kind="ExternalOutput")

    with tile.TileContext(nc) as tc:
        with tc.tile_pool(name="persist", bufs=1) as pp:
            filtT = pp.tile([P, IC, BS], F32R, tag="filtT")
            bias_sb = pp.tile([P, BC], F32, tag="bias")
            qt_sb = pp.tile([P, BC, BS], F32R, tag="qt")
            x0 = pp.tile([P, IC, TCH], F32R, tag="x0")
            nc.sync.dma_start(bias_sb[:], bias_d[:])

            with (
                tc.tile_pool(name="cayley", bufs=1) as cp,
                tc.tile_pool(name="psA", bufs=6, space="PSUM") as psA,
            ):
                # fp32 inputs that only feed DVE (HWDGE, no cast: fast start)
                eye = cp.tile([P, BC, BS], F32, tag="x", bufs=2)
                a_sb = cp.tile([P, BC, BS], F32, tag="xt", bufs=2)
                at_sb = cp.tile([P, BC, BS], F32, tag="t1t", bufs=2)
                nc.sync.dma_start(a_sb[:], pa_d[:])
                _pat_i = nc.sync.dma_start(at_sb[:], pat_d[:])
                nc.sync.dma_start(eye[:], eye_d[:])
                # prefetch during Newton (no WAR on cayley space), but
                # only after the small startup DMAs have the HBM to
                # themselves
                _x0_i = nc.gpsimd.dma_start(x0[:], xt_d[0])
                tile.add_dep_helper(
                    _x0_i.ins, _pat_i.ins, sync=True,
                    reason="defer x0 prefetch past startup DMAs")

                # The reference's 1e-6*I regularizer shifts Q by ~1e-6,
                # far below the fp32r noise floor (~3e-4), so drop it.
                # Then X1 = 2I - D = I + S = D^T and X1^T = I - S = N^T:
                # the Newton seed aliases the constant tiles, and the
                # startup DVE chain is 3 ops (s2 -> Dt -> Nt).
                s_sb = cp.tile([P, BC, BS], F32, tag="t1")
                dt_sb = cp.tile([P, BC, BS], F32R, tag="dt")  # D^T = I+S
                nc.vector.tensor_sub(s_sb[:], a_sb[:], at_sb[:])  # 2S
                nc.vector.scalar_tensor_tensor(
                    dt_sb[:], s_sb[:], 0.5, eye[:],
                    mybir.AluOpType.mult, mybir.AluOpType.add)
                nt_sb = cp.tile([P, BC, BS], F32R, tag="nt")  # N^T = I-S
                nc.vector.scalar_tensor_tensor(
                    nt_sb[:], s_sb[:], -0.5, eye[:],
                    mybir.AluOpType.mult, mybir.AluOpType.add)
                x_sb = dt_sb
                xt_sb = nt_sb

                def mm512(lhsT_tile, rhs_tile, out_sb, post=None):
                    # out = lhsT.T @ rhs for 512x512 mats in [P, BC, BS] tiles
                    for c in range(BC):
                        ps = psA.tile([P, BS], F32, tag="cay_ps")
                        for k in range(BC):
                            nc.tensor.matmul(
                                ps[:],
                                lhsT_tile[:, k, c * P:(c + 1) * P],
                                rhs_tile[:, k, :],
                                start=(k == 0),
                                stop=(k == BC - 1),
                            )
                        if post is None:
                            nc.vector.tensor_copy(out_sb[:, c, :], ps[:])
                        else:
                            post(c, ps)

                for it in range(NEWTON_ITERS):
                    t1t = cp.tile([P, BC, BS], F32R, tag="t1t", bufs=2)
                    mm512(x_sb, dt_sb, t1t)          # T1t = (D@X)^T
                    xn = cp.tile([P, BC, BS], F32R, tag="x", bufs=2)
                    xnt = cp.tile([P, BC, BS], F32R, tag="xt", bufs=2)

                    def post_xn(c, ps, _x=x_sb, _xn=xn):
                        # Xn = 2X - (DX)X
                        nc.vector.scalar_tensor_tensor(
                            _xn[:, c, :], _x[:, c, :], 2.0, ps[:],
                            mybir.AluOpType.mult, mybir.AluOpType.subtract)

                    def post_xnt(c, ps, _xt=xt_sb, _xnt=xnt):
                        nc.vector.scalar_tensor_tensor(
                            _xnt[:, c, :], _xt[:, c, :], 2.0, ps[:],
                            mybir.AluOpType.mult, mybir.AluOpType.subtract)

                    mm512(t1t, x_sb, None, post=post_xn)
                    mm512(x_sb, t1t, None, post=post_xnt)
                    x_sb, xt_sb = xn, xnt

                mm512(nt_sb, x_sb, qt_sb)       # Q^T = N @ X  (commute)

                # filt^T = W_b^T @ Q^T : lhsT = W_b (natural layout).
                # wb tiles live in the persistent region so their HWDGE
                # loads prefetch during Newton; DVE rounds to fp32r.
                _wb_dmas = []
                for g in range(IC // IGR):
                    wbr = pp.tile([P, BC, IGR * P], F32R, tag="wbr", bufs=2)
                    wb_i = nc.gpsimd.dma_start(
                        wbr[:], wb_d[:, :, g * IGR * P:(g + 1) * IGR * P])
                    _wb_dmas.append(wb_i)
                    if g < 2:
                        tile.add_dep_helper(
                            wb_i.ins, _pat_i.ins, sync=True,
                            reason="defer wb prefetch past startup DMAs")
                    for ii in range(IGR):
                        i = g * IGR + ii
                        ps = psA.tile([P, BS], F32, tag="cay_ps")
                        for k in range(BC):
                            nc.tensor.matmul(
                                ps[:],
                                wbr[:, k, ii * P:(ii + 1) * P],
                                qt_sb[:, k, :],
                                start=(k == 0),
                                stop=(k == BC - 1),
                            )
                        nc.vector.tensor_copy(filtT[:, i, :], ps[:])

            # big matmul: y^T[o,t] = filt @ x^T, accumulate over i
            with (
                tc.tile_pool(name="xstream", bufs=2) as xp,
                tc.tile_pool(name="ystage", bufs=2) as yp,
                tc.tile_pool(name="psB", bufs=6, space="PSUM") as psB,
            ):
                for t in range(NT):
                    if t == 0:
                        xtt = x0  # prefetched during Newton
                    elif t <= 2:
                        # pure SWDGE: a DVE round op here would head-of-line
                        # block the DVE stream at the Newton->filt boundary
                        xtt = xp.tile([P, IC, TCH], F32R, tag="xtile")
                        _xt_i = nc.gpsimd.dma_start(xtt[:], xt_d[t])
                        tile.add_dep_helper(
                            _xt_i.ins, _wb_dmas[-1].ins, sync=False,
                            reason="keep wb triggers ahead in SWDGE stream")
                    else:
                        xtt = xp.tile([P, IC, TCH], F32R, tag="xtile")
                        # most chunks: SWDGE cast-DMA rounds in flight
                        nc.gpsimd.dma_start(
                            xtt[:, 0:ICH, :], xt_d[t, :, 0:ICH, :])
                        # remainder: HWDGE fp32 + DVE round (path balance)
                        xst = xp.tile([P, IC - ICH, TCH], F32, tag="xstage")
                        nc.sync.dma_start(xst[:], xt_d[t, :, ICH:IC, :])
                        nc.vector.tensor_copy(xtt[:, ICH:IC, :], xst[:])
                    ys = yp.tile([P, BC, TCH], F32, tag="ys")
                    for o in range(BC):
                        ps = psB.tile([P, TCH], F32, tag="big_ps")
                        for i in range(IC):
                            nc.tensor.matmul(
                                ps[:],
                                filtT[:, i, o * P:(o + 1) * P],
                                xtt[:, i, :],
                                start=(i == 0),
                                stop=(i == IC - 1),
                            )
                        nc.scalar.activation(
                            ys[:, o, :], ps[:],
                            mybir.ActivationFunctionType.Identity,
                            bias=bias_sb[:, o:o + 1])
                        nc.sync.dma_start(yt_d[t, :, o, :], ys[:, o, :])

    nc.finalize()
    return nc


def kernel(weight, bias, x, proj_R, layer_idx=0, _trace=False, _tmpdir=None):
    weight = np.ascontiguousarray(np.asarray(weight, dtype=np.float32))
    bias = np.ascontiguousarray(np.asarray(bias, dtype=np.float32))
    x = np.ascontiguousarray(np.asarray(x, dtype=np.float32))
    proj_R = np.ascontiguousarray(np.asarray(proj_R, dtype=np.float32))

    if "nc" not in _CACHE:
        _CACHE["nc"] = _build()
    nc = _CACHE["nc"]

    def tile_pc(m):  # [BC*P, W] -> [P, BC, W] (partition-major tiling)
        return np.ascontiguousarray(
            m.reshape(BC, P, m.shape[1]).transpose(1, 0, 2))

    xt = x.reshape(NTOK, HID).T  # [HID, NTOK] view
    # [NT, P, IC, TCH]: xtl[t, p, c, j] = xt[c*P + p, t*TCH + j]
    xtl = np.ascontiguousarray(
        xt.reshape(IC, P, NT, TCH).transpose(2, 1, 0, 3))
    eye = tile_pc(np.eye(BS, dtype=np.float32))
    in_maps = []
    for b in range(NB):
        a = proj_R[b]
        in_maps.append({
            "wbl": tile_pc(weight[b * BS:(b + 1) * BS, :]),
            "pal": tile_pc(a),
            "patl": tile_pc(np.ascontiguousarray(a.T)),
            "eyel": eye,
            "bias2d": np.ascontiguousarray(
                bias[b * BS:(b + 1) * BS].reshape(BC, P).T),
            "xtl": xtl,
        })

    res = run_bass_kernel_spmd(nc, in_maps, core_ids=list(range(NB)),
                               trace=_trace, tmpdir=_tmpdir)
    out = np.empty((NTOK, HID), dtype=np.float32)
    for b in range(NB):
        # ytl[t, p, c, j] = y^T[c*P + p, t*TCH + j]
        ytb = np.ascontiguousarray(
            res.results[b]["ytl"].transpose(2, 1, 0, 3)).reshape(BS, NTOK)
        out[:, b * BS:(b + 1) * BS] = ytb.T
    if _trace:
        _CACHE["last_exec_time_ns"] = res.exec_time_ns
        _CACHE["last_results"] = res
    return out.reshape(4, 2048, HID)
